# revision 8
# baseline (speedup 1.0000x reference)
"""Trainium2 Bass kernel for the BalancedSpikingNetwork problem.

Strategy: model-parallel over neurons across 8 NeuronCores.
  - Each core owns 256 E-neurons + 64 I-neurons (padded to 384 = 3x128 rows).
  - Per step: 24 gathered spike chunks + 1 local input chunk accumulate into a
    PSUM tile = tau-scaled input currents for this core's neurons
    (batch-major). Weights are pre-scaled by tau on the host.
  - Col-tiled burst: batch=64 only fills half the PE output partitions, so
    even chunks accumulate into psum[0:64] and odd chunks into psum[64:128]
    (tile_position col groups run concurrently); a DVE add folds the halves.
  - Spikes cross cores BATCH-PAIR PACKED in fp8: byte = z[:, c] + 4*z[:, c+32]
    (values {0,1,4,5} are exact in fp8e4). This halves the AllToAll payload
    (196KB -> 98KB) and the staging DMAs. Unpack is 2 DVE ops per half
    (is_gt 3.0 for the high bit, g - 4*hi for the low bit), emitting f32r
    stationaries directly (no ACT upconvert pass).
  - Spike exchange: AllToAll with an 8x-replicated input slab (single-phase
    Mesh ~7us at 196KB; AllGather lowers to 3-stage RDH ~12.7us on this
    runtime). Replication is one SBUF->DRAM DMA with a stride-0 free dim.
  - A small PE "warmer" matmul is gated on a DMA read-back of the A2A input
    slab (which serializes behind the replication transfer on the sync
    HWDGE ring), splitting the PE-idle window during the collective so HAM
    does not re-throttle the clock to 1.2 GHz.
  - LIF update with fused scalar_tensor_tensor ops; off-critical-path state
    updates (u, t1, rates) run on GPSIMD, path ops on DVE.

The spike at step t depends only on state through t-1 (z(t) needs psum(t-1)),
so the exchange of z(t) overlaps the step-t matmul burst, and consecutive
even/odd-step chains interleave on the CC cores / PE respectively.
"""

import os
import sys

for _p in ("/opt/trn_rl_repo", "/root/.axon_site/_ro/trn_rl_repo"):
    if _p not in sys.path:
        sys.path.append(_p)

import numpy as np
import ml_dtypes

import concourse.bass as bass
import concourse.mybir as mybir
import concourse.tile as tile
from concourse import bacc
from concourse.bass_utils import run_bass_kernel_spmd
from concourse.masks import make_identity

F32 = mybir.dt.float32
F32R = mybir.dt.float32r
FP8 = mybir.dt.float8e4
OP = mybir.AluOpType

COMM = os.environ.get("COMM", "a2a")     # a2a | ag  (collective kind)
# col-tiled burst: writing matmul output to PSUM partitions 64-127 fails the
# s3d3_mm_valid_dst_partition ISA check (PE quadrant-3 HW bug), so M=64
# outputs can only target partitions 0-63 -> no second col group available.
COLT = int(os.environ.get("COLT", "0"))
WARM = int(os.environ.get("WARM", "1"))  # PE warmer on/off

B, T_FULL, IN = 64, 512, 128
N_E, N_I = 2048, 512
NCORES = 8
E_LOC = N_E // NCORES          # 256
I_LOC = N_I // NCORES          # 64
NLOC = E_LOC + I_LOC           # 320 real outputs per core
PADLOC = 384                   # padded to 3 chunks of 128
NCHUNK = PADLOC // 128         # 3 chunks per source core
GCHUNK = NCORES * NCHUNK       # 24 gathered spike chunks
KSRC = GCHUNK * 128            # 3072 gathered contraction rows
SLOT = NCHUNK * B              # 192 spike columns per core
PAIR = B // 2                  # 32 packed columns per chunk-block
PSLOT = NCHUNK * PAIR          # 96 packed columns per core
HB = GCHUNK // 2               # 12 chunks per unpack half

TAU_E = 1.0 / 20.0
TAU_I = 1.0 / 10.0
SYN_DEC = 1.0 - 1.0 / 5.0      # 0.8


def build_kernel(T: int):
    nc = bacc.Bacc(
        "TRN2", target_bir_lowering=False, debug=False, num_devices=NCORES
    )

    W_in = nc.dram_tensor("W", [KSRC, NLOC], F32R, kind="ExternalInput")
    WIN_in = nc.dram_tensor("WIN", [IN, NLOC], F32, kind="ExternalInput")
    XT_in = nc.dram_tensor("XT", [T, IN, B], F32, kind="ExternalInput")
    RATES_out = nc.dram_tensor("RATES", [B, NLOC], F32, kind="ExternalOutput")

    rg = [list(range(NCORES))]

    with tile.TileContext(nc) as tc:
        with (
            tc.tile_pool(name="persist", bufs=1) as pp,
            tc.tile_pool(name="step", bufs=2) as sp,
            tc.tile_pool(name="psum", bufs=2, space="PSUM") as psp,
            tc.tile_pool(name="tpsum", bufs=1, space="PSUM") as tpp,
            tc.tile_pool(name="dram", bufs=2, space="DRAM") as dp,
        ):
            # --- persistent tiles ---
            w_sb = pp.tile([128, GCHUNK * NLOC], F32R)            # recurrent wts
            win_sb = pp.tile([128, NLOC], F32)                    # input weights
            v_sb = pp.tile([B, NLOC], F32)                        # membrane
            u_sb = pp.tile([B, NLOC], F32)                        # tau*syn current
            zt_sb = pp.tile([128, SLOT], FP8)                     # spikes [n, b]
            zp_sb = pp.tile([128, PSLOT], FP8)                    # packed pairs
            rates_sb = pp.tile([B, NLOC], F32)                    # counts [b, n]
            ident = pp.tile([B, B], F32)
            wsb = pp.tile([1, B], FP8)                            # warmer data

            for k in range(GCHUNK):
                nc.sync.dma_start(
                    out=w_sb[:, k * NLOC : (k + 1) * NLOC],
                    in_=W_in[k * 128 : (k + 1) * 128, :],
                )
            nc.sync.dma_start(out=win_sb, in_=WIN_in[:])
            make_identity(nc, ident)
            nc.vector.memset(v_sb, 0.0)
            nc.vector.memset(u_sb, 0.0)
            nc.vector.memset(rates_sb, 0.0)

            ag_prev = None     # gathered packed spikes of step t-1
            psum_prev = None   # currents computed at step t-1
            prev_had_hi = False  # psum_prev's hi half was written

            for t in range(T):
                # ---- v_dec(t) = a*v(t-1) + 0.8*u(t-2) + psum(t-1).
                # t1 = a*v + 0.8*u uses only older state (off-path, gpsimd);
                # the psum fold is the only on-path DVE work. ----
                u08 = sp.tile([B, NLOC], F32, tag="U8")
                nc.scalar.mul(u08, u_sb, SYN_DEC)
                va = sp.tile([B, NLOC], F32, tag="VA")
                nc.scalar.mul(va[:, :E_LOC], v_sb[:, :E_LOC], 1.0 - TAU_E)
                nc.scalar.mul(va[:, E_LOC:], v_sb[:, E_LOC:], 1.0 - TAU_I)
                t1 = sp.tile([B, NLOC], F32, tag="T1")
                nc.gpsimd.tensor_tensor(out=t1, in0=va, in1=u08, op=OP.add)
                v_dec = sp.tile([B, NLOC], F32, tag="VD")
                if psum_prev is None:
                    nc.vector.tensor_copy(v_dec, t1)
                elif prev_had_hi:
                    # fold the two col-group halves through chained adds
                    # (DVE may read only ONE PSUM operand per instruction)
                    q = sp.tile([B, NLOC], F32, tag="Q")
                    nc.vector.tensor_tensor(
                        out=q, in0=t1, in1=psum_prev[0:B, :], op=OP.add
                    )
                    nc.vector.tensor_tensor(
                        out=v_dec, in0=q, in1=psum_prev[B:2 * B, :], op=OP.add
                    )
                else:
                    nc.vector.tensor_tensor(
                        out=v_dec, in0=t1, in1=psum_prev[0:B, :], op=OP.add
                    )

                # ---- spikes in [n, b] layout: 3 transposes into ONE psum
                # tile, one is_gt, one pack op ----
                tpq = tpp.tile([128, SLOT], F32, tag="TPQ")
                for j in range(NCHUNK):
                    w = 128 if j < 2 else I_LOC
                    nc.tensor.transpose(
                        tpq[:w, j * B : (j + 1) * B],
                        v_dec[:, j * 128 : j * 128 + w], ident,
                    )
                # full 128 rows: pad rows get 0/1 garbage that multiplies
                # zero weight columns (is_gt never yields NaN)
                nc.vector.tensor_scalar(
                    out=zt_sb, in0=tpq, scalar1=1.0, scalar2=None, op0=OP.is_gt,
                )
                # pack batch pairs: byte = z[:, c] + 4*z[:, c+32], exact in fp8
                zt_v = zt_sb[:].rearrange("p (j h c) -> p j h c", h=2, c=PAIR)
                nc.vector.scalar_tensor_tensor(
                    out=zp_sb[:].rearrange("p (j c) -> p j c", c=PAIR),
                    in0=zt_v[:, :, 1, :], scalar=4.0,
                    in1=zt_v[:, :, 0, :], op0=OP.mult, op1=OP.add,
                )

                # ---- exchange packed spikes (overlaps the burst below) ----
                if 1 <= t <= T - 3:
                    ag_out = dp.tile(
                        [NCORES * 128, PSLOT], FP8, tag="AGO",
                        addr_space="Local" if COMM == "a2a" else "Shared",
                    )
                    if COMM == "a2a":
                        a2a_in = dp.tile([NCORES * 128, PSLOT], FP8, tag="AGI")
                        # one-hop 8x replication: stride-0 FREE dim on
                        # the SBUF source (partition dim stays first)
                        nc.sync.dma_start(
                            out=a2a_in[:].rearrange("(d p) c -> p d c", p=128),
                            in_=zp_sb[:].unsqueeze(1).broadcast_to(
                                [128, NCORES, PSLOT]),
                        )
                        nc.gpsimd.collective_compute(
                            "AllToAll",
                            OP.bypass,
                            replica_groups=rg,
                            ins=[a2a_in[:]],
                            outs=[ag_out[:]],
                        )
                        warm_src = a2a_in
                    else:
                        ag_in = dp.tile([128, PSLOT], FP8, tag="AGI")
                        nc.sync.dma_start(out=ag_in, in_=zp_sb)
                        nc.gpsimd.collective_compute(
                            "AllGather",
                            OP.bypass,
                            replica_groups=rg,
                            ins=[ag_in[:]],
                            outs=[ag_out[:]],
                        )
                        warm_src = ag_in
                    if WARM:
                        # read-back serializes behind the staging transfer on
                        # the sync HWDGE ring -> lands mid-collective; the
                        # warmer matmul below keeps HAM at K=8/8
                        nc.sync.dma_start(out=wsb, in_=warm_src[:1, :B])
                    new_ag = ag_out
                else:
                    new_ag = None if t == 0 else ag_prev

                # ---- off-path state updates ----
                # u(t-1) = 0.8*u(t-2) + psum(t-1); recover the folded psum
                # as (v_dec - t1) so gpsimd never touches PSUM
                if psum_prev is not None:
                    u_tmp = sp.tile([B, NLOC], F32, tag="UT")
                    nc.gpsimd.tensor_tensor(
                        out=u_tmp, in0=v_dec, in1=t1, op=OP.subtract
                    )
                    nc.gpsimd.tensor_tensor(
                        out=u_sb, in0=u08, in1=u_tmp, op=OP.add
                    )
                zbn = sp.tile([B, NLOC], F32, tag="ZB")
                nc.vector.tensor_scalar(
                    out=zbn, in0=v_dec, scalar1=1.0, scalar2=None, op0=OP.is_gt
                )
                nc.gpsimd.tensor_tensor(
                    out=rates_sb, in0=rates_sb, in1=zbn, op=OP.add
                )
                # v(t) = (v_dec <= 1) * v_dec
                nc.vector.scalar_tensor_tensor(
                    out=v_sb, in0=v_dec, scalar=1.0, in1=v_dec,
                    op0=OP.is_le, op1=OP.mult,
                )

                # ---- input currents for step t (consumed at t+1) ----
                if t < T - 1:
                    sx_t = sp.tile([128, B], F32, tag="SX")
                    nc.sync.dma_start(out=sx_t, in_=XT_in[t])
                    if WARM and 1 <= t <= T - 3:
                        wps = tpp.tile([B, 8], F32, tag="WPS")
                        nc.tensor.matmul(
                            wps, wsb[:, :B], wsb[:, :8], start=True, stop=True
                        )
                    psum = psp.tile([2 * B, NLOC], F32, tag="PS")
                    nc.tensor.matmul(
                        psum[0:B, :], sx_t, win_sb,
                        start=True, stop=(ag_prev is None),
                    )
                    if ag_prev is not None:
                        # gather the packed slabs: sync takes ranks 0-3,
                        # scalar takes ranks 4-7
                        g_sb = sp.tile([128, GCHUNK * PAIR], FP8, tag="G")
                        H = NCORES // 2
                        nc.sync.dma_start(
                            out=g_sb[:, : H * PSLOT].rearrange(
                                "p (d c) -> p d c", d=H),
                            in_=ag_prev[: H * 128].rearrange(
                                "(d p) c -> p d c", p=128),
                        )
                        nc.scalar.dma_start(
                            out=g_sb[:, H * PSLOT :].rearrange(
                                "p (d c) -> p d c", d=H),
                            in_=ag_prev[H * 128 :].rearrange(
                                "(d p) c -> p d c", p=128),
                        )
                        # unpack halves to f32r stationaries
                        s_half = []
                        for hh in range(2):
                            s_t = sp.tile([128, HB * B], F32R, tag=f"S{hh}")
                            sv = s_t[:].rearrange(
                                "p (m h c) -> p m h c", h=2, c=PAIR)
                            gv = g_sb[:, hh * HB * PAIR : (hh + 1) * HB * PAIR
                                      ].rearrange("p (m c) -> p m c", c=PAIR)
                            nc.vector.tensor_scalar(
                                out=sv[:, :, 1, :], in0=gv,
                                scalar1=3.0, scalar2=None, op0=OP.is_gt,
                            )
                            nc.vector.scalar_tensor_tensor(
                                out=sv[:, :, 0, :], in0=sv[:, :, 1, :],
                                scalar=-4.0, in1=gv, op0=OP.mult, op1=OP.add,
                            )
                            s_half.append(s_t)
                        nhi = 0
                        for k in range(GCHUNK):
                            s_t = s_half[k // HB]
                            kk = k % HB
                            if COLT:
                                half = k % 2
                            else:
                                half = 0
                            nhi += half
                            nc.tensor.matmul(
                                psum[half * B : (half + 1) * B, :],
                                s_t[:, kk * B : (kk + 1) * B],
                                w_sb[:, k * NLOC : (k + 1) * NLOC],
                                start=(half == 1 and nhi == 1),
                                stop=(k >= GCHUNK - 2) if COLT
                                else (k == GCHUNK - 1),
                            )
                        had_hi = COLT == 1
                    else:
                        had_hi = False
                else:
                    psum = None
                    had_hi = False
                ag_prev = new_ag
                psum_prev = psum
                prev_had_hi = had_hi

            nc.sync.dma_start(out=RATES_out[:], in_=rates_sb[:])

    nc.compile()
    return nc


def _prep_inputs(x, W_ee, W_ie, W_ei, W_ii, W_e_in, W_i_in):
    """Host-side: combined per-core weight matrices (tau-pre-scaled) +
    transposed input."""
    Wee = np.maximum(W_ee, 0).astype(np.float32)
    Wie = np.maximum(W_ie, 0).astype(np.float32)
    Wei = np.maximum(W_ei, 0).astype(np.float32)
    Wii = np.maximum(W_ii, 0).astype(np.float32)

    Ws, Wins = [], []
    for c in range(NCORES):
        Ec = slice(c * E_LOC, (c + 1) * E_LOC)
        Ic = slice(c * I_LOC, (c + 1) * I_LOC)
        Wc = np.zeros((KSRC, NLOC), np.float32)
        for d in range(NCORES):
            base = d * PADLOC
            Epre = slice(d * E_LOC, (d + 1) * E_LOC)
            Ipre = slice(d * I_LOC, (d + 1) * I_LOC)
            Wc[base : base + E_LOC, :E_LOC] = Wee[Ec, Epre].T
            Wc[base : base + E_LOC, E_LOC:] = Wie[Ic, Epre].T
            Wc[base + E_LOC : base + NLOC, :E_LOC] = -Wei[Ec, Ipre].T
            Wc[base + E_LOC : base + NLOC, E_LOC:] = -Wii[Ic, Ipre].T
        Wc[:, :E_LOC] *= TAU_E
        Wc[:, E_LOC:] *= TAU_I
        Ws.append(Wc)

        Wi = np.empty((IN, NLOC), np.float32)
        Wi[:, :E_LOC] = W_e_in[Ec].T * TAU_E
        Wi[:, E_LOC:] = W_i_in[Ic].T * TAU_I
        Wins.append(Wi)

    xT = np.ascontiguousarray(
        np.asarray(x, np.float32).transpose(1, 2, 0)
    )  # [T, IN, B]
    return Ws, Wins, xT


_CACHE = {}


def _get_kernel(T):
    if T not in _CACHE:
        _CACHE[T] = build_kernel(T)
    return _CACHE[T]


def run_spikes(x, W_ee, W_ie, W_ei, W_ii, W_e_in, W_i_in, T=None, trace=False):
    """Run the device portion; returns spike-count sums [B, N_E] and results."""
    T = x.shape[1] if T is None else T
    Ws, Wins, xT = _prep_inputs(x, W_ee, W_ie, W_ei, W_ii, W_e_in, W_i_in)
    xT = xT[:T]
    nc = _get_kernel(T)
    in_maps = [{"W": Ws[c], "WIN": Wins[c], "XT": xT} for c in range(NCORES)]
    res = run_bass_kernel_spmd(
        nc, in_maps, core_ids=list(range(NCORES)), trace=trace
    )
    R = np.stack([res.results[c]["RATES"] for c in range(NCORES)])  # [c, b, 320]
    counts = (
        R[:, :, :E_LOC].transpose(1, 0, 2).reshape(B, N_E)
    )  # [b, c*256 + n]
    return counts, res


def kernel(x, W_ee, W_ie, W_ei, W_ii, W_e_in, W_i_in, readout_w, readout_b):
    counts, _ = run_spikes(x, W_ee, W_ie, W_ei, W_ii, W_e_in, W_i_in)
    rates = counts / np.float32(x.shape[1])
    y = rates.astype(np.float32) @ np.asarray(readout_w, np.float32).T
    return (y + np.asarray(readout_b, np.float32)).astype(np.float32)
